# revision 6
# baseline (speedup 1.0000x reference)
"""GridSample4D Trainium2 kernel (v3: single-descriptor 16-corner gather).

Sharding: 1M sample points across 8 cores (128 partitions x 1024 pts each);
the input volume is replicated as a host-precomputed fp16 lookup table.

Table layout [xs(9), ys(9), vs(269), ur(270), c(16), xx(2), yy(2), vv(2)]:
  row r = (xs,ys,vs,ur) holds 128 fp16 = 256B; a point's chunk = rows
  (r, r+1) = 512B covering all 16 corners x 16 channels:
    chunk[uu, c, xx, yy, vv] = input[c, x0+xx, y0+yy, u0+uu, v0+vv]
  with ZEROS for out-of-range corners (so no validity masks are needed --
  invalid corners contribute w*0).
  Slots: xs = x0+1, ys = y0+1, vs = v0+7, ur = u0+7 (grid in [-1.05, 1.05]
  maps to x0 in [-1,7], u0 in [-7,261]).

Per point on-device:
  coords c' = g*SC + (SC+SHIFT); slot = floor(c') via int round-trip;
  frac = c' - slot; idx = xs*C1 + ys*C2 + vs*C3 + ur; one 512B indirect
  gather; W16[uu,xx,yy,vv] = wu*wx*wy*wv (fp16); prod = chunk * W16
  (broadcast over c, fp16 2x mode); 4-level binary tree reduce (uu, xx,
  yy, vv) -> out[c] fp16.
"""
import numpy as np
from concourse import bass, mybir
from concourse.bass_utils import run_bass_kernel_spmd

P = 128
NCORES = 8
NPTS = 1024            # points per partition per core
X_, Y_, U_, V_, C_ = 8, 8, 256, 256, 16

# table geometry
XS, YS, VS, UR = 9, 9, 269, 270
R3 = XS * YS * VS * UR           # 5,883,030 rows of 128 fp16
C1 = YS * VS * UR                # 653,670
C2 = VS * UR                     # 72,630
C3 = UR                          # 270

FP32 = mybir.dt.float32
FP16 = mybir.dt.float16
I32 = mybir.dt.int32
OP = mybir.AluOpType

SC = [3.5, 3.5, 127.5, 127.5]      # d -> scale ((size-1)/2) for x,y,u,v
SHIFT = [1.0, 1.0, 7.0, 7.0]       # slot shift per dim
CMAX = [8.99, 8.99, 268.99, 268.99]  # clamp ceiling on shifted coords


def build_nc_v3(NPP=512, T=32, debug_dump=False):
    NPASS = NPTS // NPP
    NCHP = NPP // T
    NCH = NPTS // T
    nc = bass.Bass(trn_type="TRN2")
    table_d = nc.declare_dram_parameter("table", [R3, 128], FP16, isOutput=False)
    grid_d = nc.declare_dram_parameter("grid", [P, NPTS, 4], FP32, isOutput=False)
    out_d = nc.declare_dram_parameter("out", [P, NPTS, 16], FP16, isOutput=True)
    if debug_dump:
        idbg_d = nc.declare_dram_parameter("i_dbg", [P, NPP], I32, isOutput=True)
        gdbg_d = nc.declare_dram_parameter("g_dbg", [P, T, 256], FP16, isOutput=True)
        wdbg_d = nc.declare_dram_parameter("w_dbg", [P, NPP, 16], FP16, isOutput=True)

    import contextlib
    ctx = contextlib.ExitStack()
    sb = lambda name, shape, dt: ctx.enter_context(nc.sbuf_tensor(name, shape, dt))

    G = sb("G", [P, NPTS, 4], FP32)
    W16 = [sb(f"W16_{i}", [P, NPP, 16], FP16) for i in range(2)]
    IDX = [sb(f"IDX_{i}", [P, NPP], I32) for i in range(2)]
    c_t = sb("c_t", [P, NPP], FP32)
    s_t = sb("s_t", [P, NPP], FP32)
    i_t = sb("i_t", [P, NPP], I32)
    fr = [sb(f"fr{d}", [P, NPP], FP32) for d in range(4)]
    fl = [sb(f"fl{d}", [P, NPP], FP32) for d in range(4)]
    w0 = [sb(f"w0{d}", [P, NPP], FP32) for d in range(4)]
    acc = sb("acc", [P, NPP], FP32)
    A4 = sb("A4", [P, NPP, 4], FP16)
    E4 = sb("E4", [P, NPP, 4], FP16)
    gbuf = [sb(f"gbuf{i}", [P, T, 2, 16, 8], FP16) for i in range(2)]
    prod = sb("prod", [P, T, 2, 16, 8], FP16)
    L1 = sb("L1", [P, T, 16, 8], FP16)
    L2 = sb("L2", [P, T, 16, 4], FP16)
    L3 = sb("L3", [P, T, 16, 2], FP16)
    outt = [sb(f"outt{i}", [P, T, 16], FP16) for i in range(2)]

    with nc.Block() as block, \
         nc.semaphore("s_g") as s_g, \
         nc.semaphore("s_w") as s_w, \
         nc.semaphore("s_gather") as s_gather, \
         nc.semaphore("s_comb") as s_comb, \
         nc.semaphore("s_out") as s_out, \
         nc.semaphore("s_odma") as s_odma, \
         nc.semaphore("s_dbg") as s_dbg:

        @block.sync
        def _(sync):
            sync.dma_start(out=G[:], in_=grid_d[:]).then_inc(s_g, 16)
            if debug_dump:
                sync.wait_ge(s_w, 1)
                sync.dma_start(out=idbg_d[:], in_=IDX[0][:]).then_inc(s_dbg, 16)
                sync.dma_start(out=wdbg_d[:], in_=W16[0][:]).then_inc(s_dbg, 16)
                sync.wait_ge(s_gather, 16)
                sync.dma_start(
                    out=gdbg_d[:],
                    in_=gbuf[0][:].rearrange("p t a c k -> p t (a c k)"),
                ).then_inc(s_dbg, 16)
            for ch in range(NCH):
                sync.wait_ge(s_out, ch + 1)
                sync.dma_start(
                    out=out_d[:, ch * T:(ch + 1) * T, :], in_=outt[ch % 2][:]
                ).then_inc(s_odma, 16)
            sync.wait_ge(s_odma, 16 * NCH)

        @block.gpsimd
        def _(gpsimd):
            # HW firmware constraint: one indirect descriptor per partition
            # per instruction (idx[p, 0] covers the whole per-partition dst)
            # -> issue one call per point-slot.
            for ps in range(NPASS):
                gpsimd.wait_ge(s_w, ps + 1)
                for cl in range(NCHP):
                    ch = ps * NCHP + cl
                    if debug_dump and ch == 1:
                        gpsimd.wait_ge(s_dbg, 48)
                    if ch >= 2:
                        gpsimd.wait_ge(s_comb, ch - 1)
                    for t in range(T):
                        gpsimd.indirect_dma_start(
                            out=gbuf[ch % 2][:, t, :, :, :]
                                .rearrange("p a c k -> p (a c k)"),
                            out_offset=None,
                            in_=table_d[:],
                            in_offset=bass.IndirectOffsetOnAxis(
                                ap=IDX[ps % 2][:, cl * T + t:cl * T + t + 1],
                                axis=0),
                        ).then_inc(s_gather, 16)

        def weights(vector, ps):
            """Compute W16[ps%2] and IDX[ps%2] for pass ps on DVE."""
            Gs = G[:, ps * NPP:(ps + 1) * NPP, :]
            for d in range(4):
                vector.tensor_scalar(
                    out=c_t[:], in0=Gs[:, :, d], scalar1=SC[d],
                    scalar2=SC[d] + SHIFT[d], op0=OP.mult, op1=OP.add)
                vector.tensor_scalar(
                    out=c_t[:], in0=c_t[:], scalar1=0.0, scalar2=CMAX[d],
                    op0=OP.max, op1=OP.min)
                vector.tensor_scalar(
                    out=s_t[:], in0=c_t[:], scalar1=-0.5, scalar2=None,
                    op0=OP.add)
                vector.tensor_copy(out=i_t[:], in_=s_t[:])
                vector.tensor_copy(out=fl[d][:], in_=i_t[:])
                vector.tensor_tensor(
                    out=fr[d][:], in0=c_t[:], in1=fl[d][:], op=OP.subtract)
                vector.tensor_scalar(
                    out=w0[d][:], in0=fr[d][:], scalar1=-1.0, scalar2=1.0,
                    op0=OP.mult, op1=OP.add)
            # A4[(uu,xx)] = wu * wx ; E4[(yy,vv)] = wy * wv   (fp16)
            for uu in range(2):
                for xx in range(2):
                    wu = w0[2] if uu == 0 else fr[2]
                    wx = w0[0] if xx == 0 else fr[0]
                    vector.tensor_tensor(
                        out=A4[:, :, uu * 2 + xx], in0=wu[:], in1=wx[:],
                        op=OP.mult)
            for yy in range(2):
                for vv in range(2):
                    wy = w0[1] if yy == 0 else fr[1]
                    wv = w0[3] if vv == 0 else fr[3]
                    vector.tensor_tensor(
                        out=E4[:, :, yy * 2 + vv], in0=wy[:], in1=wv[:],
                        op=OP.mult)
            # W16[:, :, (uu,xx,yy,vv)] = A4[(uu,xx)] * E4[(yy,vv)]
            vector.tensor_tensor(
                out=W16[ps % 2][:].rearrange("p n (j m) -> p n j m", j=4),
                in0=A4[:].unsqueeze(3).broadcast_to([P, NPP, 4, 4]),
                in1=E4[:].unsqueeze(2).broadcast_to([P, NPP, 4, 4]),
                op=OP.mult)
            # idx = fl0*C1 + fl1*C2 + fl3*C3 + fl2
            vector.tensor_scalar(
                out=acc[:], in0=fl[0][:], scalar1=float(C1), scalar2=None,
                op0=OP.mult)
            vector.tensor_scalar(
                out=s_t[:], in0=fl[1][:], scalar1=float(C2), scalar2=None,
                op0=OP.mult)
            vector.tensor_tensor(out=acc[:], in0=acc[:], in1=s_t[:], op=OP.add)
            vector.tensor_scalar(
                out=s_t[:], in0=fl[3][:], scalar1=float(C3), scalar2=None,
                op0=OP.mult)
            vector.tensor_tensor(out=acc[:], in0=acc[:], in1=s_t[:], op=OP.add)
            vector.tensor_tensor(out=acc[:], in0=acc[:], in1=fl[2][:], op=OP.add)
            vector.tensor_copy(out=IDX[ps % 2][:], in_=acc[:]).then_inc(s_w, 1)

        @block.vector
        def _(vector):
            vector.wait_ge(s_g, 16)
            weights(vector, 0)
            for ps in range(NPASS):
                if ps + 1 < NPASS:
                    weights(vector, ps + 1)
                for cl in range(NCHP):
                    ch = ps * NCHP + cl
                    vector.wait_ge(s_gather, 16 * T * (ch + 1))
                    gb = gbuf[ch % 2]
                    for uu in range(2):
                        ins = vector.tensor_tensor(
                            out=prod[:, :, uu, :, :],
                            in0=gb[:, :, uu, :, :],
                            in1=W16[ps % 2]
                                [:, cl * T:(cl + 1) * T, uu * 8:(uu + 1) * 8]
                                .unsqueeze(2).broadcast_to([P, T, 16, 8]),
                            op=OP.mult)
                        if uu == 1:
                            ins.then_inc(s_comb, 1)
                    # tree: uu, xx, yy, vv
                    vector.tensor_tensor(
                        out=L1[:], in0=prod[:, :, 0, :, :],
                        in1=prod[:, :, 1, :, :], op=OP.add)
                    vector.tensor_tensor(
                        out=L2[:], in0=L1[:, :, :, 0:4], in1=L1[:, :, :, 4:8],
                        op=OP.add)
                    vector.tensor_tensor(
                        out=L3[:], in0=L2[:, :, :, 0:2], in1=L2[:, :, :, 2:4],
                        op=OP.add)
                    if ch >= 2:
                        vector.wait_ge(s_odma, 16 * (ch - 1))
                    vector.tensor_tensor(
                        out=outt[ch % 2][:], in0=L3[:, :, :, 0],
                        in1=L3[:, :, :, 1], op=OP.add).then_inc(s_out, 1)

    ctx.close()
    return nc


def build_table_v3(inp):
    """inp [1, C, X, Y, U, V] fp32 -> fp16 table [R3, 128]."""
    a = np.transpose(inp[0].astype(np.float16), (1, 2, 3, 4, 0))  # [x,y,u,v,c]
    Pd = np.zeros((10, 10, 270, 270, 16), np.float16)
    Pd[1:9, 1:9, 7:263, 7:263] = a
    sx, sy, su, sv, sc = Pd.strides
    from numpy.lib.stride_tricks import as_strided
    view = as_strided(
        Pd, shape=(XS, YS, VS, UR, 16, 2, 2, 2),
        strides=(sx, sy, sv, su, sc, sx, sy, sv))
    return np.ascontiguousarray(view).reshape(R3, 128)


_CACHE = {}


def prepare(inputs):
    """Build (nc, in_maps) exactly as kernel() would — for tracing."""
    input = np.asarray(inputs["input"])
    grid = np.asarray(inputs["grid"])
    ptot = P * NPTS * NCORES
    tab = build_table_v3(input)
    gpad = np.zeros((ptot, 4), np.float32)
    gpad[: grid.shape[1]] = grid[0]
    in_maps = []
    for c in range(NCORES):
        gc = gpad[c * P * NPTS:(c + 1) * P * NPTS].reshape(P, NPTS, 4)
        in_maps.append({"table": tab, "grid": np.ascontiguousarray(gc)})
    if "nc" not in _CACHE:
        _CACHE["nc"] = build_nc_v3()
    return _CACHE["nc"], in_maps


def kernel(input, grid):
    inputs = {"input": np.asarray(input), "grid": np.asarray(grid)}
    npts_in = inputs["grid"].shape[1]
    nc, in_maps = prepare(inputs)
    res = run_bass_kernel_spmd(nc, in_maps, core_ids=list(range(NCORES)))
    outs = [r["out"].reshape(P * NPTS, 16) for r in res.results]
    full = np.concatenate(outs, axis=0)[:npts_in]          # [npts, 16] fp16
    return np.ascontiguousarray(full.T[None]).astype(np.float32)


if __name__ == "__main__":
    # numeric self-check against a numpy quadrilinear reference
    rng = np.random.default_rng(0)
    inp = rng.standard_normal((1, C_, X_, Y_, U_, V_)).astype(np.float32)
    g = (rng.random((1, 3000, 4), np.float32) * 2.1 - 1.05).astype(np.float32)
    out = kernel(inp, g)
    import itertools
    sizes = (X_, Y_, U_, V_)
    coords = [(g[0, :, d] + 1) * 0.5 * (sizes[d] - 1) for d in range(4)]
    lo = [np.floor(c) for c in coords]
    fr_ = [c - l for c, l in zip(coords, lo)]
    ref = np.zeros((16, g.shape[1]), np.float32)
    inpf = inp[0].reshape(16, -1)
    for bits in itertools.product((0, 1), repeat=4):
        w = np.ones(g.shape[1], np.float32)
        valid = np.ones(g.shape[1], bool)
        idxs = []
        for d in range(4):
            i = lo[d].astype(np.int64) + bits[d]
            w = w * (fr_[d] if bits[d] else (1.0 - fr_[d]))
            valid &= (i >= 0) & (i < sizes[d])
            idxs.append(np.clip(i, 0, sizes[d] - 1))
        w = w * valid
        flat = ((idxs[0] * Y_ + idxs[1]) * U_ + idxs[2]) * V_ + idxs[3]
        ref += w * inpf[:, flat]
    err = np.abs(out[0] - ref)
    denom = np.maximum(np.abs(ref), 1e-3)
    print("max abs err:", err.max(), "max rel:", (err / denom).max())
    print("norm rel:", np.linalg.norm(out[0] - ref) / np.linalg.norm(ref))
